# revision 9
# baseline (speedup 1.0000x reference)
"""GaussianMeanShift on 8 Trainium2 NeuronCores.

Pipeline (matches reference.py semantics):
  1. k-means++-style seed selection  — tiny, sequential, jax.random-dependent:
     computed on host CPU with code identical to the reference (bit-exact).
  2. Mean-shift hill climb (10 iterations over [100 seeds] x [262144, 64]) —
     the heavy stage; runs as a Bass kernel data-parallel over points on 8
     cores with a per-iteration AllReduce of the [65,100] numerator/denominator.
  3. Connected components + per-cluster means — tiny [100,100] serial stage,
     host CPU, code identical to the reference.

Hill-climb restructure (validated to rel err ~3e-5 vs reference on the fixed
input): rows of X are unit vectors, so the row-softmax of -0.5*sqdist(Z,X)
equals the row-softmax of Z@X.T exactly up to f32 wiggle in |x_j|^2 (~1e-7).
Per core (n_loc = 32768 points):
  mm1 (PE):  G_tile[128pts, 100] = XT_tile[64, 128].T @ ZT[64, 100]
  exp (ACT): W_tile = exp(G_tile)                  (PSUM -> SBUF)
  mm2 (PE):  acc[65, 100] += Xaug_tile[128, 65].T @ W_tile[128, 100]
             (Xaug = [X | 1] so row 64 of acc is the softmax denominator)
  AllReduce acc over 8 cores; ZT_new = acc[:64] / acc[64] (broadcast via DMA).
mm1 inputs are bf16 (weight-path rounding averages out over 262k points);
mm2 stays f32 (the output is a near-cancelling mean of magnitude ~1e-4, so
the averaging path needs full precision).
"""

import os
from contextlib import ExitStack

import numpy as np
import ml_dtypes

import jax
import jax.numpy as jnp

import concourse.bass as bass
from concourse import bacc
import concourse.tile as tile
from concourse import mybir
from concourse.bass_utils import run_bass_kernel_spmd

# ---- problem constants (hardcoded per spec) ----
N = 262144
D = 64
NUM_SEEDS = 100
MAX_ITERS = 10
EPSILON = 0.05
NUM_CC = 20
N_CORES = 8
N_LOC = N // N_CORES          # 32768
NT = N_LOC // 128             # 256 k-tiles of 128 points
GROUP = 15                    # k-tiles per ACT batch (3 PSUM banks x 5)
DA = D + 1                    # 65: X augmented with a ones column

F32 = mybir.dt.float32
BF16 = mybir.dt.bfloat16

_CPU = jax.devices("cpu")[0]

# ---------------------------------------------------------------------------
# Host stages (verbatim ports of the reference, forced onto CPU jax)
# ---------------------------------------------------------------------------


def _select_smart_seeds(X, key):
    n, d = X.shape
    k0, k1 = jax.random.split(key)
    idx0 = jax.random.randint(k0, (), 0, n)
    s0 = X[idx0]
    min_d = jnp.linalg.norm(X - s0[None, :], axis=1)
    seeds = jnp.zeros((NUM_SEEDS, d), X.dtype).at[0].set(s0)
    keys = jax.random.split(k1, NUM_SEEDS - 1)

    def step(carry, xs):
        seeds, min_d = carry
        i, k = xs
        idx = jax.random.categorical(k, jnp.log(min_d))
        s = X[idx]
        dist = jnp.linalg.norm(X - s[None, :], axis=1)
        return (seeds.at[i].set(s), jnp.minimum(min_d, dist)), None

    (seeds, _), _ = jax.lax.scan(step, (seeds, min_d), (jnp.arange(1, NUM_SEEDS), keys))
    return seeds


def _sqdist(a, b):
    a2 = jnp.sum(a * a, axis=1, keepdims=True)
    b2 = jnp.sum(b * b, axis=1)[None, :]
    return jnp.maximum(a2 + b2 - 2.0 * (a @ b.T), 0.0)


def _connected_components(Z):
    m = Z.shape[0]
    Dm = jnp.sqrt(_sqdist(Z, Z))

    def body(i, carry):
        labels, K = carry
        mask = Dm[i] <= EPSILON
        labeled = mask & (labels >= 0)
        has_lab = jnp.any(labeled)
        counts = jnp.sum(jnp.where(labeled[:, None], jax.nn.one_hot(labels, m), 0.0), axis=0)
        mode = jnp.argmax(counts).astype(jnp.int32)
        new_label = jnp.where(has_lab, mode, K)
        do = labels[i] == -1
        labels = jnp.where(do & mask, new_label, labels)
        K = K + jnp.where(do & jnp.logical_not(has_lab), jnp.int32(1), jnp.int32(0))
        return labels, K

    labels, _ = jax.lax.fori_loop(0, m, body, (jnp.full((m,), -1, jnp.int32), jnp.int32(0)))
    return labels


def _finalize(Z):
    labels = _connected_components(jax.lax.stop_gradient(Z))
    onehot = jax.nn.one_hot(labels, NUM_CC, dtype=Z.dtype)
    norm = jnp.sum(onehot, axis=0)[:, None] + 1e-8
    sums = onehot.T @ Z
    return sums / norm


# ---------------------------------------------------------------------------
# Bass kernel: 10 mean-shift iterations, data-parallel over points
# ---------------------------------------------------------------------------

_NC_CACHE = None


def _build_bass():
    global _NC_CACHE
    if _NC_CACHE is not None:
        return _NC_CACHE

    # Bacc (not raw Bass): its compile() pass legalizes multi-wait
    # instructions, which this walrus rejects ("Too many sync wait commands").
    nc = bacc.Bacc("TRN2", target_bir_lowering=False, debug=False,
                   num_devices=N_CORES)

    # Per-core inputs (host pre-layouts everything; see _prep_core_inputs).
    xt_d = nc.dram_tensor("xt", [D, N_LOC], BF16, kind="ExternalInput")
    xa_d = nc.dram_tensor("xa", [128, NT * DA], F32, kind="ExternalInput")
    z0_d = nc.dram_tensor("z0", [D, NUM_SEEDS], F32, kind="ExternalInput")
    zo_d = nc.dram_tensor("zo", [D, NUM_SEEDS], F32, kind="ExternalOutput")

    # (k0, gsz) per ACT batch: 16 groups of 15 + 2 of 8 = 256.  Every group
    # has >=2 k-tiles so the second mm1 can absorb the xa-chunk DMA wait.
    groups = [(g * GROUP, GROUP) for g in range(16)] + [(240, 8), (248, 8)]

    from concourse.tile_rust import add_dep_helper

    with ExitStack() as ctx:
        tc = ctx.enter_context(tile.TileContext(nc))
        big = ctx.enter_context(tc.tile_pool(name="big", bufs=1))
        wp = ctx.enter_context(tc.tile_pool(name="wp", bufs=2))
        zp = ctx.enter_context(tc.tile_pool(name="zp", bufs=12))
        smal = ctx.enter_context(tc.tile_pool(name="smal", bufs=12))
        gp = ctx.enter_context(tc.tile_pool(name="gp", bufs=2, space="PSUM"))
        accp = ctx.enter_context(tc.tile_pool(name="accp", bufs=1, space="PSUM"))
        dram = ctx.enter_context(tc.tile_pool(name="dram", bufs=12, space="DRAM"))

        # Resident X in both layouts, loaded in group-sized chunks so the
        # first iteration's compute can start while later chunks stream in.
        xt_sb = big.tile([D, N_LOC], BF16)
        xa_sb = big.tile([128, NT * DA], F32)
        xa_dma = {}
        for k0, gsz in groups:
            nc.sync.dma_start(out=xt_sb[:, k0 * 128:(k0 + gsz) * 128],
                              in_=xt_d[:, k0 * 128:(k0 + gsz) * 128])
            xa_dma[k0] = nc.sync.dma_start(
                out=xa_sb[:, k0 * DA:(k0 + gsz) * DA],
                in_=xa_d[:, k0 * DA:(k0 + gsz) * DA])

        zt_f = zp.tile([D, NUM_SEEDS], F32, tag="ztf")
        nc.sync.dma_start(out=zt_f, in_=z0_d[:])
        zt_b = zp.tile([D, NUM_SEEDS], BF16, tag="ztb")
        nc.vector.tensor_copy(out=zt_b, in_=zt_f)

        for it in range(MAX_ITERS):
            acc = accp.tile([DA, NUM_SEEDS], F32, tag="acc")
            pending = None  # (w_tile, k0, gsz) whose mm2 is not yet emitted

            def emit_mm2(w_sb, k0, gsz, acc=acc):
                for j in range(gsz):
                    k = k0 + j
                    nc.tensor.matmul(
                        out=acc,
                        lhsT=xa_sb[:, k * DA:(k + 1) * DA],
                        rhs=w_sb[:, j * 100:(j + 1) * 100],
                        start=(k == 0),
                        stop=(k == NT - 1),
                        skip_group_check=True,
                    )

            for k0, gsz in groups:
                gps = gp.tile([128, 3, 512], F32, tag="gps")
                for j in range(gsz):
                    k = k0 + j
                    mm = nc.tensor.matmul(
                        out=gps[:, j // 5, (j % 5) * 100:(j % 5) * 100 + 100],
                        lhsT=xt_sb[:, k * 128:(k + 1) * 128],
                        rhs=zt_b,
                        start=True,
                        stop=True,
                    )
                    if it == 0 and j == 1:
                        # Pin the xa-chunk DMA wait here: this mm1 is
                        # otherwise wait-free, while the first mm2 of the
                        # group already waits on ACT — and a PE instruction
                        # only has one sync-wait slot (walrus folds Matmult
                        # waits into the LDWEIGHTS struct).
                        add_dep_helper(mm.ins, xa_dma[k0].ins, True,
                                       "pin xa-chunk wait to wait-free mm1")
                w_sb = wp.tile([128, GROUP * 100], F32, tag="w")
                nfull, rem = divmod(gsz, 5)
                if nfull:
                    nc.scalar.activation(
                        out=w_sb[:, :nfull * 500].rearrange(
                            "p (b c) -> p b c", b=nfull),
                        in_=gps[:, :nfull, :500],
                        func=mybir.ActivationFunctionType.Exp,
                    )
                if rem:
                    nc.scalar.activation(
                        out=w_sb[:, nfull * 500:nfull * 500 + rem * 100].rearrange(
                            "p (b c) -> p b c", b=1),
                        in_=gps[:, nfull:nfull + 1, :rem * 100],
                        func=mybir.ActivationFunctionType.Exp,
                    )
                if pending is not None:
                    emit_mm2(*pending)
                pending = (w_sb, k0, gsz)
            emit_mm2(*pending)

            # Cross-core reduction of [numerator | denominator].
            nm_sb = smal.tile([DA, NUM_SEEDS], F32, tag="nm")
            nc.vector.tensor_copy(out=nm_sb, in_=acc)
            nm_dr = dram.tile([DA, NUM_SEEDS], F32, tag="nmd")
            nc.sync.dma_start(out=nm_dr, in_=nm_sb)
            rd_dr = dram.tile([DA, NUM_SEEDS], F32, tag="rdd", addr_space="Shared")
            nc.gpsimd.collective_compute(
                "AllReduce",
                mybir.AluOpType.add,
                replica_groups=[list(range(N_CORES))],
                ins=[nm_dr.opt()],
                outs=[rd_dr.opt()],
            )
            r_sb = smal.tile([D, NUM_SEEDS], F32, tag="r")
            nc.sync.dma_start(out=r_sb, in_=rd_dr[0:D, :])
            den_sb = smal.tile([D, NUM_SEEDS], F32, tag="den")
            nc.sync.dma_start(
                out=den_sb,
                in_=rd_dr[D:DA, :].partition_broadcast(D),
            )
            rec_sb = smal.tile([D, NUM_SEEDS], F32, tag="rec")
            nc.vector.reciprocal(out=rec_sb, in_=den_sb)
            zt_f = zp.tile([D, NUM_SEEDS], F32, tag="ztf")
            nc.vector.tensor_mul(out=zt_f, in0=r_sb, in1=rec_sb)
            zt_b = zp.tile([D, NUM_SEEDS], BF16, tag="ztb")
            nc.vector.tensor_copy(out=zt_b, in_=zt_f)

        nc.sync.dma_start(out=zo_d[:], in_=zt_f)

    nc.compile()
    _NC_CACHE = nc
    return nc


def _prep_core_inputs(X, seeds):
    """X: [N, D] f32, seeds: [NUM_SEEDS, D] f32 -> list of per-core in_maps."""
    z0 = np.ascontiguousarray(seeds.T.astype(np.float32))  # [D, 100]
    in_maps = []
    for c in range(N_CORES):
        shard = X[c * N_LOC:(c + 1) * N_LOC]  # [N_LOC, D]
        xt = np.ascontiguousarray(shard.T).astype(ml_dtypes.bfloat16)
        xa = np.concatenate(
            [shard.reshape(NT, 128, D),
             np.ones((NT, 128, 1), np.float32)], axis=2)      # [NT, 128, DA]
        xa = np.ascontiguousarray(xa.transpose(1, 0, 2).reshape(128, NT * DA))
        in_maps.append({"xt": xt, "xa": xa, "z0": z0})
    return in_maps


LAST_RESULT = None  # BassKernelResults of the most recent kernel() call


def kernel(X: np.ndarray) -> np.ndarray:
    global LAST_RESULT
    X = np.ascontiguousarray(np.asarray(X, dtype=np.float32))
    assert X.shape == (N, D)

    # Stage 1: seed selection on CPU jax (bit-identical to reference).
    with jax.default_device(_CPU):
        Xj = jnp.asarray(X)
        seeds = np.asarray(_select_smart_seeds(Xj, jax.random.key(42)))

    # Stage 2: hill climb on 8 NeuronCores.
    nc = _build_bass()
    in_maps = _prep_core_inputs(X, seeds)
    res = run_bass_kernel_spmd(
        nc,
        in_maps,
        core_ids=list(range(N_CORES)),
        trace=bool(int(os.environ.get("KERNEL_TRACE", "0"))),
    )
    LAST_RESULT = res
    zt = res.results[0]["zo"]                    # [D, 100] f32
    Z = np.ascontiguousarray(zt.T)               # [100, D]

    # Stage 3: connected components + cluster means on CPU jax.
    with jax.default_device(_CPU):
        out = np.asarray(_finalize(jnp.asarray(Z)))
    return out.astype(np.float32)


if __name__ == "__main__":
    X = np.random.default_rng(0).standard_normal((N, D)).astype(np.float32)
    X /= np.linalg.norm(X, axis=1, keepdims=True)
    out = kernel(X)
    print(out.shape, out.dtype, np.abs(out).max())


# revision 14
# speedup vs baseline: 91.8676x; 91.8676x over previous
"""GaussianMeanShift on 8 Trainium2 NeuronCores.

Pipeline (matches reference.py semantics):
  1. k-means++-style seed selection  — tiny, sequential, jax.random-dependent:
     computed on host CPU with code identical to the reference (bit-exact).
  2. Mean-shift hill climb (10 iterations over [100 seeds] x [262144, 64]) —
     the heavy stage; runs as a Bass kernel data-parallel over points on 8
     cores with a per-iteration AllReduce of the [65,100] numerator/denominator.
  3. Connected components + per-cluster means — tiny [100,100] serial stage,
     host CPU, code identical to the reference.

Hill-climb restructure (validated to rel err ~3e-5 vs reference on the fixed
input): rows of X are unit vectors, so the row-softmax of -0.5*sqdist(Z,X)
equals the row-softmax of Z@X.T exactly up to f32 wiggle in |x_j|^2 (~1e-7).
Per core (n_loc = 32768 points):
  mm1 (PE):  G_tile[128pts, 100] = XT_tile[64, 128].T @ ZT[64, 100]
  exp (ACT): W_tile = exp(G_tile)                  (PSUM -> SBUF)
  mm2 (PE):  acc[65, 100] += Xaug_tile[128, 65].T @ W_tile[128, 100]
             (Xaug = [X | 1] so row 64 of acc is the softmax denominator)
  AllReduce acc over 8 cores; ZT_new = acc[:64] / acc[64] (broadcast via DMA).
mm1 inputs are bf16 (weight-path rounding averages out over 262k points);
mm2 stays f32 (the output is a near-cancelling mean of magnitude ~1e-4, so
the averaging path needs full precision).
"""

import os
from contextlib import ExitStack

import numpy as np
import ml_dtypes

import jax
import jax.numpy as jnp

import concourse.bass as bass
from concourse import bacc
import concourse.tile as tile
from concourse import mybir
from concourse.bass_utils import run_bass_kernel_spmd

# ---- problem constants (hardcoded per spec) ----
N = 262144
D = 64
NUM_SEEDS = 100
MAX_ITERS = 10
EPSILON = 0.05
NUM_CC = 20
N_CORES = 8
N_LOC = N // N_CORES          # 32768
NT = N_LOC // 128             # 256 k-tiles of 128 points
GROUP = 15                    # k-tiles per ACT batch (3 PSUM banks x 5)
DA = D + 1                    # 65: X augmented with a ones column

F32 = mybir.dt.float32
BF16 = mybir.dt.bfloat16

_CPU = jax.devices("cpu")[0]

# ---------------------------------------------------------------------------
# Host stages (verbatim ports of the reference, forced onto CPU jax)
# ---------------------------------------------------------------------------


def _select_smart_seeds(X, key):
    n, d = X.shape
    k0, k1 = jax.random.split(key)
    idx0 = jax.random.randint(k0, (), 0, n)
    s0 = X[idx0]
    min_d = jnp.linalg.norm(X - s0[None, :], axis=1)
    seeds = jnp.zeros((NUM_SEEDS, d), X.dtype).at[0].set(s0)
    keys = jax.random.split(k1, NUM_SEEDS - 1)

    def step(carry, xs):
        seeds, min_d = carry
        i, k = xs
        idx = jax.random.categorical(k, jnp.log(min_d))
        s = X[idx]
        dist = jnp.linalg.norm(X - s[None, :], axis=1)
        return (seeds.at[i].set(s), jnp.minimum(min_d, dist)), None

    (seeds, _), _ = jax.lax.scan(step, (seeds, min_d), (jnp.arange(1, NUM_SEEDS), keys))
    return seeds


def _sqdist(a, b):
    a2 = jnp.sum(a * a, axis=1, keepdims=True)
    b2 = jnp.sum(b * b, axis=1)[None, :]
    return jnp.maximum(a2 + b2 - 2.0 * (a @ b.T), 0.0)


def _connected_components(Z):
    m = Z.shape[0]
    Dm = jnp.sqrt(_sqdist(Z, Z))

    def body(i, carry):
        labels, K = carry
        mask = Dm[i] <= EPSILON
        labeled = mask & (labels >= 0)
        has_lab = jnp.any(labeled)
        counts = jnp.sum(jnp.where(labeled[:, None], jax.nn.one_hot(labels, m), 0.0), axis=0)
        mode = jnp.argmax(counts).astype(jnp.int32)
        new_label = jnp.where(has_lab, mode, K)
        do = labels[i] == -1
        labels = jnp.where(do & mask, new_label, labels)
        K = K + jnp.where(do & jnp.logical_not(has_lab), jnp.int32(1), jnp.int32(0))
        return labels, K

    labels, _ = jax.lax.fori_loop(0, m, body, (jnp.full((m,), -1, jnp.int32), jnp.int32(0)))
    return labels


def _finalize(Z):
    labels = _connected_components(jax.lax.stop_gradient(Z))
    onehot = jax.nn.one_hot(labels, NUM_CC, dtype=Z.dtype)
    norm = jnp.sum(onehot, axis=0)[:, None] + 1e-8
    sums = onehot.T @ Z
    return sums / norm


# ---------------------------------------------------------------------------
# Bass kernel: 10 mean-shift iterations, data-parallel over points
# ---------------------------------------------------------------------------

_NC_CACHE = None


def _build_bass(sim_single_core=False, iters=MAX_ITERS, skip_mm2=False):
    global _NC_CACHE
    if not sim_single_core and _NC_CACHE is not None:
        return _NC_CACHE

    # Bacc (not raw Bass): its compile() pass legalizes multi-wait
    # instructions, which this walrus rejects ("Too many sync wait commands").
    nc = bacc.Bacc("TRN2", target_bir_lowering=False, debug=False,
                   num_devices=1 if sim_single_core else N_CORES)

    # Per-core inputs (host pre-layouts everything; see _prep_core_inputs).
    xt_d = nc.dram_tensor("xt", [D, N_LOC], BF16, kind="ExternalInput")
    xa_d = nc.dram_tensor("xa", [128, NT * DA], F32, kind="ExternalInput")
    z0_d = nc.dram_tensor("z0", [D, NUM_SEEDS], F32, kind="ExternalInput")
    zo_d = nc.dram_tensor("zo", [D, NUM_SEEDS], F32, kind="ExternalOutput")

    # (k0, gsz) per ACT batch: 16 groups of 15 + 2 of 8 = 256.  Every group
    # has >=2 k-tiles so the second mm1 can absorb the xa-chunk DMA wait.
    groups = [(g * GROUP, GROUP) for g in range(16)] + [(240, 8), (248, 8)]

    from concourse.tile_rust import add_dep_helper

    with ExitStack() as ctx:
        tc = ctx.enter_context(tile.TileContext(nc))
        big = ctx.enter_context(tc.tile_pool(name="big", bufs=1))
        wp = ctx.enter_context(tc.tile_pool(name="wp", bufs=3))
        zp = ctx.enter_context(tc.tile_pool(name="zp", bufs=12))
        smal = ctx.enter_context(tc.tile_pool(name="smal", bufs=12))
        gp = ctx.enter_context(tc.tile_pool(name="gp", bufs=2, space="PSUM"))
        accp = ctx.enter_context(tc.tile_pool(name="accp", bufs=1, space="PSUM"))
        dram = ctx.enter_context(tc.tile_pool(name="dram", bufs=12, space="DRAM"))

        # Resident X in both layouts, loaded in group-sized chunks so the
        # first iteration's compute can start while later chunks stream in.
        xt_sb = big.tile([D, N_LOC], BF16)
        xa_sb = big.tile([128, NT * DA], F32)
        xa_dma = {}
        for k0, gsz in groups:
            nc.sync.dma_start(out=xt_sb[:, k0 * 128:(k0 + gsz) * 128],
                              in_=xt_d[:, k0 * 128:(k0 + gsz) * 128])
            xa_dma[k0] = nc.sync.dma_start(
                out=xa_sb[:, k0 * DA:(k0 + gsz) * DA],
                in_=xa_d[:, k0 * DA:(k0 + gsz) * DA])

        zt_f = zp.tile([D, NUM_SEEDS], F32, tag="ztf")
        nc.sync.dma_start(out=zt_f, in_=z0_d[:])
        zt_b = zp.tile([D, NUM_SEEDS], BF16, tag="ztb")
        nc.vector.tensor_copy(out=zt_b, in_=zt_f)

        for it in range(iters):
            acc = accp.tile([DA, NUM_SEEDS], F32, tag="acc")
            pending = None  # (w_tile, k0, gsz) whose mm2 is not yet emitted

            def emit_mm2(w_sb, k0, gsz, acc=acc):
                for j in range(gsz):
                    k = k0 + j
                    nc.tensor.matmul(
                        out=acc,
                        lhsT=xa_sb[:, k * DA:(k + 1) * DA],
                        rhs=w_sb[:, j * 100:(j + 1) * 100],
                        start=(k == 0),
                        stop=(k == NT - 1),
                        skip_group_check=True,
                    )

            for k0, gsz in groups:
                gps = gp.tile([128, 3, 512], F32, tag="gps")
                for j in range(gsz):
                    k = k0 + j
                    mm = nc.tensor.matmul(
                        out=gps[:, j // 5, (j % 5) * 100:(j % 5) * 100 + 100],
                        lhsT=xt_sb[:, k * 128:(k + 1) * 128],
                        rhs=zt_b,
                        start=True,
                        stop=True,
                    )
                    if it == 0 and j == 1:
                        # Pin the xa-chunk DMA wait here: this mm1 is
                        # otherwise wait-free, while the first mm2 of the
                        # group already waits on ACT — and a PE instruction
                        # only has one sync-wait slot (walrus folds Matmult
                        # waits into the LDWEIGHTS struct).
                        add_dep_helper(mm.ins, xa_dma[k0].ins, True,
                                       "pin xa-chunk wait to wait-free mm1")
                w_sb = wp.tile([128, GROUP * 100], F32, tag="w")
                nfull, rem = divmod(gsz, 5)
                if nfull:
                    nc.scalar.activation(
                        out=w_sb[:, :nfull * 500].rearrange(
                            "p (b c) -> p b c", b=nfull),
                        in_=gps[:, :nfull, :500],
                        func=mybir.ActivationFunctionType.Exp,
                    )
                if rem:
                    nc.scalar.activation(
                        out=w_sb[:, nfull * 500:nfull * 500 + rem * 100].rearrange(
                            "p (b c) -> p b c", b=1),
                        in_=gps[:, nfull:nfull + 1, :rem * 100],
                        func=mybir.ActivationFunctionType.Exp,
                    )
                if pending is not None and not skip_mm2:
                    emit_mm2(*pending)
                pending = (w_sb, k0, gsz)
            if not skip_mm2:
                emit_mm2(*pending)

            # Cross-core reduction of [numerator | denominator].
            nm_sb = smal.tile([DA, NUM_SEEDS], F32, tag="nm")
            nc.vector.tensor_copy(out=nm_sb, in_=acc)
            nm_dr = dram.tile([DA, NUM_SEEDS], F32, tag="nmd")
            nc.sync.dma_start(out=nm_dr, in_=nm_sb)
            rd_dr = dram.tile([DA, NUM_SEEDS], F32, tag="rdd", addr_space="Shared")
            if sim_single_core:
                # TimelineSim can't model collectives; stand in a DRAM copy.
                nc.sync.dma_start(out=rd_dr, in_=nm_dr)
            else:
                nc.gpsimd.collective_compute(
                    "AllReduce",
                    mybir.AluOpType.add,
                    replica_groups=[list(range(N_CORES))],
                    ins=[nm_dr.opt()],
                    outs=[rd_dr.opt()],
                )
            r_sb = smal.tile([D, NUM_SEEDS], F32, tag="r")
            nc.sync.dma_start(out=r_sb, in_=rd_dr[0:D, :])
            den_sb = smal.tile([D, NUM_SEEDS], F32, tag="den")
            nc.sync.dma_start(
                out=den_sb,
                in_=rd_dr[D:DA, :].partition_broadcast(D),
            )
            rec_sb = smal.tile([D, NUM_SEEDS], F32, tag="rec")
            nc.vector.reciprocal(out=rec_sb, in_=den_sb)
            zt_f = zp.tile([D, NUM_SEEDS], F32, tag="ztf")
            nc.vector.tensor_mul(out=zt_f, in0=r_sb, in1=rec_sb)
            zt_b = zp.tile([D, NUM_SEEDS], BF16, tag="ztb")
            nc.vector.tensor_copy(out=zt_b, in_=zt_f)

        nc.sync.dma_start(out=zo_d[:], in_=zt_f)

    nc.compile()
    if not sim_single_core:
        _NC_CACHE = nc
    return nc


def _prep_core_inputs(X, seeds):
    """X: [N, D] f32, seeds: [NUM_SEEDS, D] f32 -> list of per-core in_maps."""
    z0 = np.ascontiguousarray(seeds.T.astype(np.float32))  # [D, 100]
    in_maps = []
    for c in range(N_CORES):
        shard = X[c * N_LOC:(c + 1) * N_LOC]  # [N_LOC, D]
        xt = np.ascontiguousarray(shard.T).astype(ml_dtypes.bfloat16)
        xa = np.concatenate(
            [shard.reshape(NT, 128, D),
             np.ones((NT, 128, 1), np.float32)], axis=2)      # [NT, 128, DA]
        xa = np.ascontiguousarray(xa.transpose(1, 0, 2).reshape(128, NT * DA))
        in_maps.append({"xt": xt, "xa": xa, "z0": z0})
    return in_maps


LAST_RESULT = None  # BassKernelResults of the most recent kernel() call


def kernel(X: np.ndarray) -> np.ndarray:
    global LAST_RESULT
    X = np.ascontiguousarray(np.asarray(X, dtype=np.float32))
    assert X.shape == (N, D)

    # Stage 1: seed selection on CPU jax (bit-identical to reference).
    with jax.default_device(_CPU):
        Xj = jnp.asarray(X)
        seeds = np.asarray(_select_smart_seeds(Xj, jax.random.key(42)))

    # Stage 2: hill climb on 8 NeuronCores.
    nc = _build_bass()
    in_maps = _prep_core_inputs(X, seeds)
    res = run_bass_kernel_spmd(
        nc,
        in_maps,
        core_ids=list(range(N_CORES)),
        trace=bool(int(os.environ.get("KERNEL_TRACE", "0"))),
    )
    LAST_RESULT = res
    zt = res.results[0]["zo"]                    # [D, 100] f32
    Z = np.ascontiguousarray(zt.T)               # [100, D]

    # Stage 3: connected components + cluster means on CPU jax.
    with jax.default_device(_CPU):
        out = np.asarray(_finalize(jnp.asarray(Z)))
    return out.astype(np.float32)


if __name__ == "__main__":
    X = np.random.default_rng(0).standard_normal((N, D)).astype(np.float32)
    X /= np.linalg.norm(X, axis=1, keepdims=True)
    out = kernel(X)
    print(out.shape, out.dtype, np.abs(out).max())


# revision 15
# speedup vs baseline: 92.5177x; 1.0071x over previous
"""GaussianMeanShift on 8 Trainium2 NeuronCores.

Pipeline (matches reference.py semantics):
  1. k-means++-style seed selection  — tiny, sequential, jax.random-dependent:
     computed on host CPU with code identical to the reference (bit-exact).
  2. Mean-shift hill climb (10 iterations over [100 seeds] x [262144, 64]) —
     the heavy stage; runs as a Bass kernel data-parallel over points on 8
     cores with a per-iteration AllReduce of the [65,100] numerator/denominator.
  3. Connected components + per-cluster means — tiny [100,100] serial stage,
     host CPU, code identical to the reference.

Hill-climb restructure (validated to rel err ~3e-5 vs reference on the fixed
input): rows of X are unit vectors, so the row-softmax of -0.5*sqdist(Z,X)
equals the row-softmax of Z@X.T exactly up to f32 wiggle in |x_j|^2 (~1e-7).
Per core (n_loc = 32768 points):
  mm1 (PE):  G_tile[128pts, 100] = XT_tile[64, 128].T @ ZT[64, 100]
  exp (ACT): W_tile = exp(G_tile)                  (PSUM -> SBUF)
  mm2 (PE):  acc[65, 100] += Xaug_tile[128, 65].T @ W_tile[128, 100]
             (Xaug = [X | 1] so row 64 of acc is the softmax denominator)
  AllReduce acc over 8 cores; ZT_new = acc[:64] / acc[64] (broadcast via DMA).
mm1 inputs are bf16 (weight-path rounding averages out over 262k points);
mm2 stays f32 (the output is a near-cancelling mean of magnitude ~1e-4, so
the averaging path needs full precision).
"""

import os
from contextlib import ExitStack

import numpy as np
import ml_dtypes

import jax
import jax.numpy as jnp

import concourse.bass as bass
from concourse import bacc
import concourse.tile as tile
from concourse import mybir
from concourse.bass_utils import run_bass_kernel_spmd

# ---- problem constants (hardcoded per spec) ----
N = 262144
D = 64
NUM_SEEDS = 100
MAX_ITERS = 10
EPSILON = 0.05
NUM_CC = 20
N_CORES = 8
N_LOC = N // N_CORES          # 32768
NT = N_LOC // 128             # 256 k-tiles of 128 points
GROUP = 15                    # k-tiles per ACT batch (3 PSUM banks x 5)
DA = D + 1                    # 65: X augmented with a ones column

F32 = mybir.dt.float32
BF16 = mybir.dt.bfloat16
FP8 = mybir.dt.float8e4

_CPU = jax.devices("cpu")[0]

# ---------------------------------------------------------------------------
# Host stages (verbatim ports of the reference, forced onto CPU jax)
# ---------------------------------------------------------------------------


def _select_smart_seeds(X, key):
    n, d = X.shape
    k0, k1 = jax.random.split(key)
    idx0 = jax.random.randint(k0, (), 0, n)
    s0 = X[idx0]
    min_d = jnp.linalg.norm(X - s0[None, :], axis=1)
    seeds = jnp.zeros((NUM_SEEDS, d), X.dtype).at[0].set(s0)
    keys = jax.random.split(k1, NUM_SEEDS - 1)

    def step(carry, xs):
        seeds, min_d = carry
        i, k = xs
        idx = jax.random.categorical(k, jnp.log(min_d))
        s = X[idx]
        dist = jnp.linalg.norm(X - s[None, :], axis=1)
        return (seeds.at[i].set(s), jnp.minimum(min_d, dist)), None

    (seeds, _), _ = jax.lax.scan(step, (seeds, min_d), (jnp.arange(1, NUM_SEEDS), keys))
    return seeds


def _sqdist(a, b):
    a2 = jnp.sum(a * a, axis=1, keepdims=True)
    b2 = jnp.sum(b * b, axis=1)[None, :]
    return jnp.maximum(a2 + b2 - 2.0 * (a @ b.T), 0.0)


def _connected_components(Z):
    m = Z.shape[0]
    Dm = jnp.sqrt(_sqdist(Z, Z))

    def body(i, carry):
        labels, K = carry
        mask = Dm[i] <= EPSILON
        labeled = mask & (labels >= 0)
        has_lab = jnp.any(labeled)
        counts = jnp.sum(jnp.where(labeled[:, None], jax.nn.one_hot(labels, m), 0.0), axis=0)
        mode = jnp.argmax(counts).astype(jnp.int32)
        new_label = jnp.where(has_lab, mode, K)
        do = labels[i] == -1
        labels = jnp.where(do & mask, new_label, labels)
        K = K + jnp.where(do & jnp.logical_not(has_lab), jnp.int32(1), jnp.int32(0))
        return labels, K

    labels, _ = jax.lax.fori_loop(0, m, body, (jnp.full((m,), -1, jnp.int32), jnp.int32(0)))
    return labels


def _finalize(Z):
    labels = _connected_components(jax.lax.stop_gradient(Z))
    onehot = jax.nn.one_hot(labels, NUM_CC, dtype=Z.dtype)
    norm = jnp.sum(onehot, axis=0)[:, None] + 1e-8
    sums = onehot.T @ Z
    return sums / norm


# ---------------------------------------------------------------------------
# Bass kernel: 10 mean-shift iterations, data-parallel over points
# ---------------------------------------------------------------------------

_NC_CACHE = None


def _build_bass(sim_single_core=False, iters=MAX_ITERS, skip_mm2=False):
    global _NC_CACHE
    if not sim_single_core and _NC_CACHE is not None:
        return _NC_CACHE

    # Bacc (not raw Bass): its compile() pass legalizes multi-wait
    # instructions, which this walrus rejects ("Too many sync wait commands").
    nc = bacc.Bacc("TRN2", target_bir_lowering=False, debug=False,
                   num_devices=1 if sim_single_core else N_CORES)

    # Per-core inputs (host pre-layouts everything; see _prep_core_inputs).
    xt_d = nc.dram_tensor("xt", [D, N_LOC], FP8, kind="ExternalInput")
    xa_d = nc.dram_tensor("xa", [128, NT * DA], F32, kind="ExternalInput")
    z0_d = nc.dram_tensor("z0", [D, NUM_SEEDS], F32, kind="ExternalInput")
    zo_d = nc.dram_tensor("zo", [D, NUM_SEEDS], F32, kind="ExternalOutput")

    # (k0, gsz) per ACT batch: 16 groups of 15 + 2 of 8 = 256.  Every group
    # has >=2 k-tiles so the second mm1 can absorb the xa-chunk DMA wait.
    groups = [(g * GROUP, GROUP) for g in range(16)] + [(240, 8), (248, 8)]

    from concourse.tile_rust import add_dep_helper

    with ExitStack() as ctx:
        tc = ctx.enter_context(tile.TileContext(nc))
        big = ctx.enter_context(tc.tile_pool(name="big", bufs=1))
        wp = ctx.enter_context(tc.tile_pool(name="wp", bufs=3))
        zp = ctx.enter_context(tc.tile_pool(name="zp", bufs=12))
        smal = ctx.enter_context(tc.tile_pool(name="smal", bufs=12))
        gp = ctx.enter_context(tc.tile_pool(name="gp", bufs=2, space="PSUM"))
        accp = ctx.enter_context(tc.tile_pool(name="accp", bufs=1, space="PSUM"))
        dram = ctx.enter_context(tc.tile_pool(name="dram", bufs=12, space="DRAM"))

        # Resident X in both layouts, loaded in group-sized chunks so the
        # first iteration's compute can start while later chunks stream in.
        xt_sb = big.tile([D, N_LOC], FP8)
        xa_sb = big.tile([128, NT * DA], F32)
        xa_dma = {}
        for k0, gsz in groups:
            nc.sync.dma_start(out=xt_sb[:, k0 * 128:(k0 + gsz) * 128],
                              in_=xt_d[:, k0 * 128:(k0 + gsz) * 128])
            xa_dma[k0] = nc.sync.dma_start(
                out=xa_sb[:, k0 * DA:(k0 + gsz) * DA],
                in_=xa_d[:, k0 * DA:(k0 + gsz) * DA])

        zt_f = zp.tile([D, NUM_SEEDS], F32, tag="ztf")
        nc.sync.dma_start(out=zt_f, in_=z0_d[:])
        zt_b = zp.tile([D, NUM_SEEDS], BF16, tag="ztb")
        nc.vector.tensor_copy(out=zt_b, in_=zt_f)

        for it in range(iters):
            acc = accp.tile([DA, NUM_SEEDS], F32, tag="acc")
            pending = None  # (w_tile, k0, gsz) whose mm2 is not yet emitted

            def emit_mm2(w_sb, k0, gsz, acc=acc):
                for j in range(gsz):
                    k = k0 + j
                    nc.tensor.matmul(
                        out=acc,
                        lhsT=xa_sb[:, k * DA:(k + 1) * DA],
                        rhs=w_sb[:, j * 100:(j + 1) * 100],
                        start=(k == 0),
                        stop=(k == NT - 1),
                        skip_group_check=True,
                    )

            for k0, gsz in groups:
                gps = gp.tile([128, 3, 512], F32, tag="gps")
                for j in range(gsz):
                    k = k0 + j
                    mm = nc.tensor.matmul(
                        out=gps[:, j // 5, (j % 5) * 100:(j % 5) * 100 + 100],
                        lhsT=xt_sb[:, k * 128:(k + 1) * 128],
                        rhs=zt_b,
                        start=True,
                        stop=True,
                    )
                    if it == 0 and j == 1:
                        # Pin the xa-chunk DMA wait here: this mm1 is
                        # otherwise wait-free, while the first mm2 of the
                        # group already waits on ACT — and a PE instruction
                        # only has one sync-wait slot (walrus folds Matmult
                        # waits into the LDWEIGHTS struct).
                        add_dep_helper(mm.ins, xa_dma[k0].ins, True,
                                       "pin xa-chunk wait to wait-free mm1")
                w_sb = wp.tile([128, GROUP * 100], F32, tag="w")
                nfull, rem = divmod(gsz, 5)
                if nfull:
                    nc.scalar.activation(
                        out=w_sb[:, :nfull * 500].rearrange(
                            "p (b c) -> p b c", b=nfull),
                        in_=gps[:, :nfull, :500],
                        func=mybir.ActivationFunctionType.Exp,
                    )
                if rem:
                    nc.scalar.activation(
                        out=w_sb[:, nfull * 500:nfull * 500 + rem * 100].rearrange(
                            "p (b c) -> p b c", b=1),
                        in_=gps[:, nfull:nfull + 1, :rem * 100],
                        func=mybir.ActivationFunctionType.Exp,
                    )
                if pending is not None and not skip_mm2:
                    emit_mm2(*pending)
                pending = (w_sb, k0, gsz)
            if not skip_mm2:
                emit_mm2(*pending)

            # Cross-core reduction of [numerator | denominator].
            nm_sb = smal.tile([DA, NUM_SEEDS], F32, tag="nm")
            nc.vector.tensor_copy(out=nm_sb, in_=acc)
            nm_dr = dram.tile([DA, NUM_SEEDS], F32, tag="nmd")
            nc.sync.dma_start(out=nm_dr, in_=nm_sb)
            rd_dr = dram.tile([DA, NUM_SEEDS], F32, tag="rdd", addr_space="Shared")
            if sim_single_core:
                # TimelineSim can't model collectives; stand in a DRAM copy.
                nc.sync.dma_start(out=rd_dr, in_=nm_dr)
            else:
                nc.gpsimd.collective_compute(
                    "AllReduce",
                    mybir.AluOpType.add,
                    replica_groups=[list(range(N_CORES))],
                    ins=[nm_dr.opt()],
                    outs=[rd_dr.opt()],
                )
            r_sb = smal.tile([D, NUM_SEEDS], F32, tag="r")
            nc.sync.dma_start(out=r_sb, in_=rd_dr[0:D, :])
            den_sb = smal.tile([D, NUM_SEEDS], F32, tag="den")
            nc.sync.dma_start(
                out=den_sb,
                in_=rd_dr[D:DA, :].partition_broadcast(D),
            )
            rec_sb = smal.tile([D, NUM_SEEDS], F32, tag="rec")
            nc.vector.reciprocal(out=rec_sb, in_=den_sb)
            zt_f = zp.tile([D, NUM_SEEDS], F32, tag="ztf")
            nc.vector.tensor_mul(out=zt_f, in0=r_sb, in1=rec_sb)
            zt_b = zp.tile([D, NUM_SEEDS], BF16, tag="ztb")
            nc.vector.tensor_copy(out=zt_b, in_=zt_f)

        nc.sync.dma_start(out=zo_d[:], in_=zt_f)

    nc.compile()
    if not sim_single_core:
        _NC_CACHE = nc
    return nc


def _prep_core_inputs(X, seeds):
    """X: [N, D] f32, seeds: [NUM_SEEDS, D] f32 -> list of per-core in_maps."""
    z0 = np.ascontiguousarray(seeds.T.astype(np.float32))  # [D, 100]
    in_maps = []
    for c in range(N_CORES):
        shard = X[c * N_LOC:(c + 1) * N_LOC]  # [N_LOC, D]
        xt = np.ascontiguousarray(shard.T).astype(ml_dtypes.float8_e4m3)
        xa = np.concatenate(
            [shard.reshape(NT, 128, D),
             np.ones((NT, 128, 1), np.float32)], axis=2)      # [NT, 128, DA]
        xa = np.ascontiguousarray(xa.transpose(1, 0, 2).reshape(128, NT * DA))
        in_maps.append({"xt": xt, "xa": xa, "z0": z0})
    return in_maps


LAST_RESULT = None  # BassKernelResults of the most recent kernel() call


def kernel(X: np.ndarray) -> np.ndarray:
    global LAST_RESULT
    X = np.ascontiguousarray(np.asarray(X, dtype=np.float32))
    assert X.shape == (N, D)

    # Stage 1: seed selection on CPU jax (bit-identical to reference).
    with jax.default_device(_CPU):
        Xj = jnp.asarray(X)
        seeds = np.asarray(_select_smart_seeds(Xj, jax.random.key(42)))

    # Stage 2: hill climb on 8 NeuronCores.
    nc = _build_bass()
    in_maps = _prep_core_inputs(X, seeds)
    res = run_bass_kernel_spmd(
        nc,
        in_maps,
        core_ids=list(range(N_CORES)),
        trace=bool(int(os.environ.get("KERNEL_TRACE", "0"))),
    )
    LAST_RESULT = res
    zt = res.results[0]["zo"]                    # [D, 100] f32
    Z = np.ascontiguousarray(zt.T)               # [100, D]

    # Stage 3: connected components + cluster means on CPU jax.
    with jax.default_device(_CPU):
        out = np.asarray(_finalize(jnp.asarray(Z)))
    return out.astype(np.float32)


if __name__ == "__main__":
    X = np.random.default_rng(0).standard_normal((N, D)).astype(np.float32)
    X /= np.linalg.norm(X, axis=1, keepdims=True)
    out = kernel(X)
    print(out.shape, out.dtype, np.abs(out).max())
